# revision 11
# baseline (speedup 1.0000x reference)
"""Trainium2 Bass kernel for the thin-plate-spline RBF layer.

reference:  out[b,n,d] = sum_m phi(|x_bn - c_bm|) * w[b,m,d],
            phi(r) = r^2 * log(r + 1e-6)

Device algorithm (per core, N sharded 8 ways), scalar-LN-paced pipeline:
  dist2[m,n] = sum_k a_k[m] * b_k[n]   -- rank-15 bf16 split-precision
      expansion of |x-c|^2 (coordinates centered, split into bf16 hi/lo;
      bf16 products are exact under fp32 PSUM accumulation).  The four
      batches sit on four 32-row PE strips (tile_position row tiling),
      two strips per [128, 1024] PSUM tile, and run concurrently.
  L[m,n] = ln(dist2 + 5e-5)            (ScalarE Ln, fp32 out -> SBUF)
      The ScalarE stream (32 x 1024-col ACTIVATEs ~ 1us each) is the
      kernel's pacing resource; everything else hides under it.
  The elementwise dist2*L multiply is eliminated algebraically:
    out[b,n,d] = sum_k b_k[n] * S[(k,b,d), n],
    S = sum_m (0.5 * a_k[m] * w[m,d]) * L[m,n]   (TensorE fp32,
        8 accumulating matmuls per n-tile, 60 used of 64 columns).
  z = S * bcs (DVE), o2 = rmat^T z (TensorE, overwriting rows 0:12 of
  the same PSUM bank as S once z has drained it), DVE copy to SBUF,
  DMA out.
  Emission interleaves S-chain matmuls of tile t-1 between the dist2
  matmul groups of tile t so the TensorE FIFO never starves ScalarE.
"""
import sys

sys.path.insert(0, "/opt/trn_rl_repo")

import numpy as np
import ml_dtypes

BF16 = np.dtype(ml_dtypes.bfloat16)

B, M, N, NCORES = 4, 256, 32768, 8
NS = N // NCORES          # 4096 dense points per core
NT = 512                  # n-tile (one PSUM bank of fp32)
NTILES = NS // NT         # 8
HALVES = M // 128         # 2
NBLK = B * HALVES         # 8 contraction blocks of 128
KD = 15                   # dist2 split-precision rank
J = 5 * B * 3             # 60 S rows, j = k*12 + b*3 + d
DELTA = 5e-5

_compiled = None


def _build_nc():
    import concourse.bacc as bacc
    import concourse.mybir as mybir
    from concourse.tile import TileContext

    f32 = mybir.dt.float32
    f32r = mybir.dt.float32r
    bf = mybir.dt.bfloat16
    nc = bacc.Bacc("TRN2")

    daug_d = nc.dram_tensor("daug", [128, NS], bf, kind="ExternalInput")
    cpa_d = nc.dram_tensor("cpa", [128, HALVES * 128], bf, kind="ExternalInput")
    wps_d = nc.dram_tensor("wps", [128, NBLK * 64], f32r, kind="ExternalInput")
    bcs_d = nc.dram_tensor("bcs", [J, NS], f32, kind="ExternalInput")
    rmat_d = nc.dram_tensor("rmat", [J, 32], f32r, kind="ExternalInput")
    out_d = nc.dram_tensor("outb", [12, NS], f32, kind="ExternalOutput")

    GROUPS = [(0, 0), (0, 1), (1, 0), (1, 1)]   # (h, i) emission order

    with TileContext(nc) as tc:
        with (
            tc.tile_pool(name="singles", bufs=1) as singles,
            tc.tile_pool(name="lpool", bufs=10) as lpool,
            tc.tile_pool(name="zpool", bufs=3) as zpool,
            tc.tile_pool(name="d2pool", bufs=3, space="PSUM") as d2pool,
            tc.tile_pool(name="spool", bufs=2, space="PSUM") as spool,
        ):
            delta_t = singles.tile([128, 1], f32)
            nc.vector.memset(delta_t, DELTA)
            scratch = singles.tile([128, NT], bf)
            nc.vector.memset(scratch[:], 0.0)

            # --- input DMAs, most-urgent first, spread over two queues ---
            cpa_t = singles.tile([128, HALVES * 128], bf)
            nc.sync.dma_start(out=cpa_t[:], in_=cpa_d[:])
            daug_t = singles.tile([128, NS], bf)
            QN = NS // 4
            nc.sync.dma_start(out=daug_t[:, 0:QN], in_=daug_d[:, 0:QN])
            wps_t = singles.tile([128, NBLK * 64], f32r)
            nc.sync.dma_start(out=wps_t[:], in_=wps_d[:])
            nc.sync.dma_start(out=daug_t[:, QN : 2 * QN], in_=daug_d[:, QN : 2 * QN])
            rmat_t = singles.tile([J, 32], f32r)
            nc.gpsimd.dma_start(out=rmat_t[:], in_=rmat_d[:])
            nc.gpsimd.dma_start(out=daug_t[:, 2 * QN : 3 * QN],
                                in_=daug_d[:, 2 * QN : 3 * QN])
            nc.gpsimd.dma_start(out=daug_t[:, 3 * QN :], in_=daug_d[:, 3 * QN :])
            bcs_t = singles.tile([J, NS], f32)
            nc.gpsimd.dma_start(out=bcs_t[:, : NS // 2], in_=bcs_d[:, : NS // 2])
            nc.gpsimd.dma_start(out=bcs_t[:, NS // 2 :], in_=bcs_d[:, NS // 2 :])
            out_sb = singles.tile([12, NS], f32)

            # HAM warmup on junk data while the input DMAs land
            wtile = d2pool.tile([128, 2 * NT], f32, tag="d2")
            for _ in range(4):
                nc.tensor.matmul(
                    wtile[:, :NT], scratch[:, :128], scratch[:],
                    start=True, stop=True,
                )

            ltiles = {}
            s_tiles = {}

            def emit_s_pair(u, g):
                """Two S-chain matmuls for tile u, group g (pairs with the
                L tile (u, GROUPS[g]))."""
                h, i = GROUPS[g]
                lt = ltiles[(u, h, i)]
                if g == 0:
                    s_tiles[u] = spool.tile([128, NT], f32, tag="s",
                                            name=f"s{u}")
                s_c = s_tiles[u]
                for bi in range(2):
                    b = 2 * i + bi
                    l = 2 * b + h
                    nc.tensor.matmul(
                        s_c[0:64, :],
                        wps_t[:, l * 64 : (l + 1) * 64],
                        lt[:, bi * NT : (bi + 1) * NT],
                        start=(g == 0 and bi == 0),
                        stop=(g == 3 and bi == 1),
                    )

            def emit_combine(u):
                """z-mult, o2 reduction, copy-out, output DMA for tile u."""
                nsl = slice(u * NT, (u + 1) * NT)
                s_c = s_tiles[u]
                z_t = zpool.tile([J, NT], f32r, tag="z")
                nc.vector.tensor_mul(z_t[:], s_c[0:J, :], bcs_t[:, nsl])
                nc.tensor.matmul(s_c[0:12, :], rmat_t[:, 0:12], z_t[:],
                                 start=True, stop=True)
                nc.vector.tensor_copy(out_sb[:, nsl], s_c[0:12, :])
                if u % 2 == 1:
                    osl = slice((u - 1) * NT, (u + 1) * NT)
                    nc.sync.dma_start(out=out_d[:, osl], in_=out_sb[:, osl])

            for t in range(NTILES):
                nsl = slice(t * NT, (t + 1) * NT)
                for g, (h, i) in enumerate(GROUPS):
                    if t > 0:
                        emit_s_pair(t - 1, g)
                    d2 = d2pool.tile([128, 2 * NT], f32, tag="d2")
                    for bi in range(2):
                        b = 2 * i + bi
                        nc.tensor.matmul(
                            d2[:, bi * NT : (bi + 1) * NT],
                            cpa_t[32 * b : 32 * b + KD,
                                  h * 128 : (h + 1) * 128],
                            daug_t[32 * b : 32 * b + KD, nsl],
                            start=True,
                            stop=True,
                            tile_position=(32 * b, 0),
                        )
                    lt = lpool.tile([128, 2 * NT], f32r, tag="L")
                    nc.scalar.activation(
                        out=lt[:],
                        in_=d2[:],
                        func=mybir.ActivationFunctionType.Ln,
                        bias=delta_t[:],
                        scale=1.0,
                    )
                    ltiles[(t, h, i)] = lt
                if t > 0:
                    emit_combine(t - 1)

            # drain the last tile
            for g in range(4):
                emit_s_pair(NTILES - 1, g)
            emit_combine(NTILES - 1)

    nc.compile()
    return nc


def _split3(v):
    """3-way bf16 split of float64 array."""
    hi = v.astype(BF16)
    r1 = v - hi.astype(np.float64)
    mid = r1.astype(BF16)
    r2 = r1 - mid.astype(np.float64)
    lo = r2.astype(BF16)
    return hi, mid, lo


def _host_prep(sparse_disp, original_cp, original_dense):
    """Build per-core input maps for the device kernel."""
    x = original_dense.astype(np.float64) - 0.5   # (B, N, 3) centered
    c = original_cp.astype(np.float64) - 0.5      # (B, M, 3)
    w = sparse_disp.astype(np.float32)            # (B, M, 3)

    # ---- control-point side (shared by all cores) ----
    p = c.astype(BF16)
    q = (c - p.astype(np.float64)).astype(BF16)
    t_hi, t_mid, t_lo = _split3((c * c).sum(-1))
    ones_m = np.ones((B, M), BF16)

    # per-batch KD rows: [p x3, p x3, q x3, t_hi, t_mid, t_lo, 1, 1, 1]
    cpa_full = np.empty((B, KD, M), BF16)
    for d in range(3):
        cpa_full[:, d, :] = p[:, :, d]
        cpa_full[:, 3 + d, :] = p[:, :, d]
        cpa_full[:, 6 + d, :] = q[:, :, d]
    cpa_full[:, 9, :] = t_hi
    cpa_full[:, 10, :] = t_mid
    cpa_full[:, 11, :] = t_lo
    cpa_full[:, 12, :] = ones_m
    cpa_full[:, 13, :] = ones_m
    cpa_full[:, 14, :] = ones_m

    # stacked stationary: rows 32b..32b+KD, cols h*128..
    cpa = np.zeros((128, HALVES * 128), BF16)
    for b in range(B):
        for h in range(HALVES):
            cpa[32 * b : 32 * b + KD, h * 128 : (h + 1) * 128] = \
                cpa_full[b, :, h * 128 : (h + 1) * 128]

    # S-chain stationaries, one 64-col block per l = 2*b + h, bf16
    wps = np.zeros((128, NBLK * 64), np.float32)
    c32 = c.astype(np.float32)
    a5 = np.stack(
        [c32[:, :, 0], c32[:, :, 1], c32[:, :, 2],
         (c32 * c32).sum(-1), np.ones((B, M), np.float32)],
        axis=1,
    )  # (B, 5, M)
    for b in range(B):
        for h in range(HALVES):
            l = 2 * b + h
            msl = slice(h * 128, (h + 1) * 128)
            for k in range(5):
                for d in range(3):
                    j = k * 12 + b * 3 + d
                    wps[:, l * 64 + j] = 0.5 * a5[b, k, msl] * w[b, msl, d]

    rmat = np.zeros((J, 32), np.float32)
    for j in range(J):
        rmat[j, j % 12] = 1.0

    # ---- dense-point side (per core) ----
    u_all = x.astype(BF16)
    v_all = (x - u_all.astype(np.float64)).astype(BF16)
    s_all = (x * x).sum(-1)

    in_maps = []
    for core in range(NCORES):
        csl = slice(core * NS, (core + 1) * NS)
        u = u_all[:, csl, :].astype(np.float32)
        v = v_all[:, csl, :].astype(np.float32)
        s_hi, s_mid, s_lo = _split3(s_all[:, csl])
        ones_n = np.ones((B, NS), BF16)

        daug_b = np.empty((B, KD, NS), BF16)
        for d in range(3):
            daug_b[:, d, :] = (-2.0 * u[:, :, d]).astype(BF16)
            daug_b[:, 3 + d, :] = (-2.0 * v[:, :, d]).astype(BF16)
            daug_b[:, 6 + d, :] = (-2.0 * u[:, :, d]).astype(BF16)
        daug_b[:, 9, :] = ones_n
        daug_b[:, 10, :] = ones_n
        daug_b[:, 11, :] = ones_n
        daug_b[:, 12, :] = s_hi
        daug_b[:, 13, :] = s_mid
        daug_b[:, 14, :] = s_lo

        daug = np.zeros((128, NS), BF16)
        for b in range(B):
            daug[32 * b : 32 * b + KD] = daug_b[b]

        xs = x[:, csl, :].astype(np.float32)
        baug5 = np.stack(
            [-2.0 * xs[:, :, 0], -2.0 * xs[:, :, 1], -2.0 * xs[:, :, 2],
             np.ones((B, NS), np.float32), (xs * xs).sum(-1)],
            axis=1,
        )  # (B, 5, NS)
        bc = np.empty((J, NS), np.float32)
        for k in range(5):
            for b in range(B):
                for d in range(3):
                    bc[k * 12 + b * 3 + d] = baug5[b, k]

        in_maps.append(
            {
                "daug": daug,
                "bcs": bc,
                "cpa": cpa,
                "wps": wps,
                "rmat": rmat,
            }
        )
    return in_maps


def _assemble(results):
    out = np.empty((B, N, 3), np.float32)
    for core, r in enumerate(results):
        o = r["outb"]  # (12, NS) rows b*3+d
        out[:, core * NS : (core + 1) * NS, :] = (
            o.reshape(B, 3, NS).transpose(0, 2, 1)
        )
    return out


def kernel(sparse_disp, original_cp, original_dense):
    global _compiled
    from concourse.bass_utils import run_bass_kernel_spmd

    if _compiled is None:
        _compiled = _build_nc()
    in_maps = _host_prep(sparse_disp, original_cp, original_dense)
    res = run_bass_kernel_spmd(_compiled, in_maps, core_ids=list(range(NCORES)))
    return _assemble(res.results)


# revision 12
# speedup vs baseline: 1.2447x; 1.2447x over previous
"""Trainium2 Bass kernel for the thin-plate-spline RBF layer.

reference:  out[b,n,d] = sum_m phi(|x_bn - c_bm|) * w[b,m,d],
            phi(r) = r^2 * log(r + 1e-6)

Device algorithm (per core, N sharded 8 ways), scalar-LN-paced pipeline:
  dist2[m,n] = sum_k a_k[m] * b_k[n]   -- rank-15 bf16 split-precision
      expansion of |x-c|^2 (coordinates centered, split into bf16 hi/lo;
      bf16 products are exact under fp32 PSUM accumulation).  The four
      batches sit on four 32-row PE strips (tile_position row tiling),
      two strips per [128, 1024] PSUM tile, and run concurrently.
  L[m,n] = ln(dist2 + 5e-5)            (ScalarE Ln, fp32 out -> SBUF)
      The ScalarE stream (32 x 1024-col ACTIVATEs ~ 1us each) is the
      kernel's pacing resource; everything else hides under it.
  The elementwise dist2*L multiply is eliminated algebraically:
    out[b,n,d] = sum_k b_k[n] * S[(k,b,d), n],
    S = sum_m (0.5 * a_k[m] * w[m,d]) * L[m,n]   (TensorE fp32,
        8 accumulating matmuls per n-tile, 60 used of 64 columns).
  z = S * bcs (DVE), o2 = rmat^T z (TensorE, overwriting rows 0:12 of
  the same PSUM bank as S once z has drained it), DVE copy to SBUF,
  DMA out.
  Emission interleaves S-chain matmuls of tile t-1 between the dist2
  matmul groups of tile t so the TensorE FIFO never starves ScalarE.
"""
import sys

sys.path.insert(0, "/opt/trn_rl_repo")

import numpy as np
import ml_dtypes

BF16 = np.dtype(ml_dtypes.bfloat16)

B, M, N, NCORES = 4, 256, 32768, 8
NS = N // NCORES          # 4096 dense points per core
NT = 512                  # n-tile (one PSUM bank of fp32)
NTILES = NS // NT         # 8
HALVES = M // 128         # 2
NBLK = B * HALVES         # 8 contraction blocks of 128
KD = 15                   # dist2 split-precision rank
J = 5 * B * 3             # 60 S rows, j = k*12 + b*3 + d
DELTA = 5e-5

_compiled = None


def _build_nc():
    import concourse.bacc as bacc
    import concourse.mybir as mybir
    from concourse.tile import TileContext

    f32 = mybir.dt.float32
    f32r = mybir.dt.float32r
    bf = mybir.dt.bfloat16
    nc = bacc.Bacc("TRN2")

    daug_d = nc.dram_tensor("daug", [128, NS], bf, kind="ExternalInput")
    cpa_d = nc.dram_tensor("cpa", [128, HALVES * 128], bf, kind="ExternalInput")
    wps_d = nc.dram_tensor("wps", [128, NBLK * 120], bf, kind="ExternalInput")
    bcs_d = nc.dram_tensor("bcs", [J, NS], f32, kind="ExternalInput")
    rmat_d = nc.dram_tensor("rmat", [2 * J, 12], f32r, kind="ExternalInput")
    out_d = nc.dram_tensor("outb", [12, NS], f32, kind="ExternalOutput")

    GROUPS = [(0, 0), (0, 1), (1, 0), (1, 1)]   # (h, i) emission order

    with TileContext(nc) as tc:
        with (
            tc.tile_pool(name="singles", bufs=1) as singles,
            tc.tile_pool(name="lpool", bufs=10) as lpool,
            tc.tile_pool(name="zpool", bufs=3) as zpool,
            tc.tile_pool(name="d2pool", bufs=3, space="PSUM") as d2pool,
            tc.tile_pool(name="spool", bufs=2, space="PSUM") as spool,
        ):
            delta_t = singles.tile([128, 1], f32)
            nc.vector.memset(delta_t, DELTA)
            scratch = singles.tile([128, NT], bf)
            nc.vector.memset(scratch[:], 0.0)

            # --- input DMAs, most-urgent first, spread over two queues ---
            cpa_t = singles.tile([128, HALVES * 128], bf)
            nc.sync.dma_start(out=cpa_t[:], in_=cpa_d[:])
            daug_t = singles.tile([128, NS], bf)
            QN = NS // 4
            nc.sync.dma_start(out=daug_t[:, 0:QN], in_=daug_d[:, 0:QN])
            wps_t = singles.tile([128, NBLK * 120], bf)
            nc.sync.dma_start(out=wps_t[:], in_=wps_d[:])
            nc.sync.dma_start(out=daug_t[:, QN : 2 * QN], in_=daug_d[:, QN : 2 * QN])
            rmat_t = singles.tile([2 * J, 12], f32r)
            nc.gpsimd.dma_start(out=rmat_t[:], in_=rmat_d[:])
            nc.gpsimd.dma_start(out=daug_t[:, 2 * QN : 3 * QN],
                                in_=daug_d[:, 2 * QN : 3 * QN])
            nc.gpsimd.dma_start(out=daug_t[:, 3 * QN :], in_=daug_d[:, 3 * QN :])
            bcs_t = singles.tile([2 * J, NS], f32)
            nc.gpsimd.dma_start(out=bcs_t[0:J, :], in_=bcs_d[:])
            nc.gpsimd.dma_start(out=bcs_t[J : 2 * J, :], in_=bcs_d[:])
            out_sb = singles.tile([12, NS], f32)

            # HAM warmup on junk data while the input DMAs land
            wtile = d2pool.tile([128, 2 * NT], f32, tag="d2")
            for _ in range(5):
                nc.tensor.matmul(
                    wtile[:, :NT], scratch[:, :128], scratch[:],
                    start=True, stop=True,
                )

            ltiles = {}
            s_tiles = {}

            def emit_s_pair(u, g):
                """Two S-chain matmuls for tile u, group g (pairs with the
                L tile (u, GROUPS[g]))."""
                h, i = GROUPS[g]
                lt = ltiles[(u, h, i)]
                if g == 0:
                    s_tiles[u] = spool.tile([128, NT], f32, tag="s",
                                            name=f"s{u}")
                s_c = s_tiles[u]
                for bi in range(2):
                    b = 2 * i + bi
                    l = 2 * b + h
                    nc.tensor.matmul(
                        s_c[0 : 2 * J, :],
                        wps_t[:, l * 120 : (l + 1) * 120],
                        lt[:, bi * NT : (bi + 1) * NT],
                        start=(g == 0 and bi == 0),
                        stop=(g == 3 and bi == 1),
                    )

            def emit_combine(u):
                """z-mult, o2 reduction, copy-out, output DMA for tile u."""
                nsl = slice(u * NT, (u + 1) * NT)
                s_c = s_tiles[u]
                z_t = zpool.tile([2 * J, NT], f32r, tag="z")
                nc.vector.tensor_mul(z_t[:], s_c[0 : 2 * J, :], bcs_t[:, nsl])
                nc.tensor.matmul(s_c[0:12, :], rmat_t[:], z_t[:],
                                 start=True, stop=True)
                nc.vector.tensor_copy(out_sb[:, nsl], s_c[0:12, :])
                if u % 2 == 1:
                    osl = slice((u - 1) * NT, (u + 1) * NT)
                    nc.sync.dma_start(out=out_d[:, osl], in_=out_sb[:, osl])

            for t in range(NTILES):
                nsl = slice(t * NT, (t + 1) * NT)
                for g, (h, i) in enumerate(GROUPS):
                    if t > 0:
                        emit_s_pair(t - 1, g)
                    d2 = d2pool.tile([128, 2 * NT], f32, tag="d2")
                    for bi in range(2):
                        b = 2 * i + bi
                        nc.tensor.matmul(
                            d2[:, bi * NT : (bi + 1) * NT],
                            cpa_t[32 * b : 32 * b + KD,
                                  h * 128 : (h + 1) * 128],
                            daug_t[32 * b : 32 * b + KD, nsl],
                            start=True,
                            stop=True,
                            tile_position=(32 * b, 0),
                        )
                    lt = lpool.tile([128, 2 * NT], bf, tag="L")
                    nc.scalar.activation(
                        out=lt[:],
                        in_=d2[:],
                        func=mybir.ActivationFunctionType.Ln,
                        bias=delta_t[:],
                        scale=1.0,
                    )
                    ltiles[(t, h, i)] = lt
                if t > 0:
                    emit_combine(t - 1)

            # drain the last tile
            for g in range(4):
                emit_s_pair(NTILES - 1, g)
            emit_combine(NTILES - 1)

    nc.compile()
    return nc


def _split3(v):
    """3-way bf16 split of float64 array."""
    hi = v.astype(BF16)
    r1 = v - hi.astype(np.float64)
    mid = r1.astype(BF16)
    r2 = r1 - mid.astype(np.float64)
    lo = r2.astype(BF16)
    return hi, mid, lo


def _host_prep(sparse_disp, original_cp, original_dense):
    """Build per-core input maps for the device kernel."""
    x = original_dense.astype(np.float64) - 0.5   # (B, N, 3) centered
    c = original_cp.astype(np.float64) - 0.5      # (B, M, 3)
    w = sparse_disp.astype(np.float32)            # (B, M, 3)

    # ---- control-point side (shared by all cores) ----
    p = c.astype(BF16)
    q = (c - p.astype(np.float64)).astype(BF16)
    t_hi, t_mid, t_lo = _split3((c * c).sum(-1))
    ones_m = np.ones((B, M), BF16)

    # per-batch KD rows: [p x3, p x3, q x3, t_hi, t_mid, t_lo, 1, 1, 1]
    cpa_full = np.empty((B, KD, M), BF16)
    for d in range(3):
        cpa_full[:, d, :] = p[:, :, d]
        cpa_full[:, 3 + d, :] = p[:, :, d]
        cpa_full[:, 6 + d, :] = q[:, :, d]
    cpa_full[:, 9, :] = t_hi
    cpa_full[:, 10, :] = t_mid
    cpa_full[:, 11, :] = t_lo
    cpa_full[:, 12, :] = ones_m
    cpa_full[:, 13, :] = ones_m
    cpa_full[:, 14, :] = ones_m

    # stacked stationary: rows 32b..32b+KD, cols h*128..
    cpa = np.zeros((128, HALVES * 128), BF16)
    for b in range(B):
        for h in range(HALVES):
            cpa[32 * b : 32 * b + KD, h * 128 : (h + 1) * 128] = \
                cpa_full[b, :, h * 128 : (h + 1) * 128]

    # S-chain stationaries: per l = 2*b + h a [128, 120] block whose
    # cols 0:60 hold bf16-hi and 60:120 bf16-lo of the fp32 weights
    wpsf = np.zeros((128, NBLK, J), np.float32)
    c32 = c.astype(np.float32)
    a5 = np.stack(
        [c32[:, :, 0], c32[:, :, 1], c32[:, :, 2],
         (c32 * c32).sum(-1), np.ones((B, M), np.float32)],
        axis=1,
    )  # (B, 5, M)
    for b in range(B):
        for h in range(HALVES):
            l = 2 * b + h
            msl = slice(h * 128, (h + 1) * 128)
            for k in range(5):
                for d in range(3):
                    j = k * 12 + b * 3 + d
                    wpsf[:, l, j] = 0.5 * a5[b, k, msl] * w[b, msl, d]
    wps_hi = wpsf.astype(BF16)
    wps_lo = (wpsf - wps_hi.astype(np.float32)).astype(BF16)
    wps = np.zeros((128, NBLK * 120), BF16)
    for l in range(NBLK):
        wps[:, l * 120 : l * 120 + J] = wps_hi[:, l, :]
        wps[:, l * 120 + J : (l + 1) * 120] = wps_lo[:, l, :]

    rmat = np.zeros((2 * J, 12), np.float32)
    for j in range(J):
        rmat[j, j % 12] = 1.0
        rmat[J + j, j % 12] = 1.0

    # ---- dense-point side (per core) ----
    u_all = x.astype(BF16)
    v_all = (x - u_all.astype(np.float64)).astype(BF16)
    s_all = (x * x).sum(-1)

    in_maps = []
    for core in range(NCORES):
        csl = slice(core * NS, (core + 1) * NS)
        u = u_all[:, csl, :].astype(np.float32)
        v = v_all[:, csl, :].astype(np.float32)
        s_hi, s_mid, s_lo = _split3(s_all[:, csl])
        ones_n = np.ones((B, NS), BF16)

        daug_b = np.empty((B, KD, NS), BF16)
        for d in range(3):
            daug_b[:, d, :] = (-2.0 * u[:, :, d]).astype(BF16)
            daug_b[:, 3 + d, :] = (-2.0 * v[:, :, d]).astype(BF16)
            daug_b[:, 6 + d, :] = (-2.0 * u[:, :, d]).astype(BF16)
        daug_b[:, 9, :] = ones_n
        daug_b[:, 10, :] = ones_n
        daug_b[:, 11, :] = ones_n
        daug_b[:, 12, :] = s_hi
        daug_b[:, 13, :] = s_mid
        daug_b[:, 14, :] = s_lo

        daug = np.zeros((128, NS), BF16)
        for b in range(B):
            daug[32 * b : 32 * b + KD] = daug_b[b]

        xs = x[:, csl, :].astype(np.float32)
        baug5 = np.stack(
            [-2.0 * xs[:, :, 0], -2.0 * xs[:, :, 1], -2.0 * xs[:, :, 2],
             np.ones((B, NS), np.float32), (xs * xs).sum(-1)],
            axis=1,
        )  # (B, 5, NS)
        bc = np.empty((J, NS), np.float32)
        for k in range(5):
            for b in range(B):
                for d in range(3):
                    bc[k * 12 + b * 3 + d] = baug5[b, k]

        in_maps.append(
            {
                "daug": daug,
                "bcs": bc,
                "cpa": cpa,
                "wps": wps,
                "rmat": rmat,
            }
        )
    return in_maps


def _assemble(results):
    out = np.empty((B, N, 3), np.float32)
    for core, r in enumerate(results):
        o = r["outb"]  # (12, NS) rows b*3+d
        out[:, core * NS : (core + 1) * NS, :] = (
            o.reshape(B, 3, NS).transpose(0, 2, 1)
        )
    return out


def kernel(sparse_disp, original_cp, original_dense):
    global _compiled
    from concourse.bass_utils import run_bass_kernel_spmd

    if _compiled is None:
        _compiled = _build_nc()
    in_maps = _host_prep(sparse_disp, original_cp, original_dense)
    res = run_bass_kernel_spmd(_compiled, in_maps, core_ids=list(range(NCORES)))
    return _assemble(res.results)
